# revision 6
# baseline (speedup 1.0000x reference)
"""Q8 linear layer (dequant matmul) on 8 Trainium2 NeuronCores.

out[t, o] = sum_i (x[t, i] * scales[i]) * weight[o, i]

Sharding: tensor-parallel over out_features (14336 = 8 * 1792). Each core
receives the full (pre-scaled, pre-transposed) activations and a 1792-column
slice of weight^T, computes its [32, 1792] f32 output slice, and the host
concatenates. int8-valued weights are exact in bf16.

Strategies (A/B tested on HW):
  deliver="bf16":  weight shipped as bf16, plain HWDGE DMA (2 B/elem HBM).
  deliver="cast":  weight shipped as int8, SWDGE cast-DMA int8->bf16
                   (1 B/elem HBM read, 2 B/elem SBUF write).
  matmul="serial": 128 matmuls [128,32]x[128,448] accumulating in 4 banks.
  matmul="packed4": 4 k-tiles run concurrently in the 4 PE column groups
                   (tile_position), partial sums folded with 2 DVE adds.
"""

import os
import sys

for _p in ("/opt/trn_rl_repo", "/root/.axon_site/_ro/trn_rl_repo"):
    if os.path.isdir(_p) and _p not in sys.path:
        sys.path.insert(0, _p)

import numpy as np
import ml_dtypes

import concourse.bass as bass
import concourse.mybir as mybir
import concourse.tile as tile
from concourse import bacc
from concourse.bass_utils import run_bass_kernel_spmd

TOKENS = 32
IN_F = 4096
OUT_F = 14336
NCORES = 8
OPC = OUT_F // NCORES  # 1792 out features per core
KT = IN_F // 128  # 32 k-tiles
KG = 4  # k-tiles per weight DMA group
NG = KT // KG  # 8 weight DMA groups
OB = 4  # output column blocks per core
OBS = OPC // OB  # 448 columns per block (fits one PSUM bank)

DELIVER = "bf16"
MATMUL = "serial"

_cached_nc = {}


def _emit_body(nc, pools, aps, deliver, matmul, it=0):
    xpool, wpool, opool, pspool = pools
    xsT_r, wT_r, out = aps

    xs_sb = xpool.tile(
        [128, KT, TOKENS], mybir.dt.bfloat16, name=f"xs_sb_{it}", tag="xs_sb"
    )
    nc.sync.dma_start(out=xs_sb[:], in_=xsT_r)

    # weight tiles: [128, KG, OPC] bf16, one DMA per group of KG k-tiles
    w_tiles = []
    for g in range(NG):
        w_sb = wpool.tile(
            [128, KG, OPC], mybir.dt.bfloat16, name=f"w_sb{it}_{g}", tag="w_sb"
        )
        if deliver == "cast":
            nc.gpsimd.dma_start(out=w_sb[:], in_=wT_r[g])  # SWDGE int8->bf16
        else:
            nc.sync.dma_start(out=w_sb[:], in_=wT_r[g])
        w_tiles.append(w_sb)

    def w_ap(ki, ob):
        return w_tiles[ki // KG][:, ki % KG, ob * OBS : (ob + 1) * OBS]

    out_sb = opool.tile(
        [TOKENS, OPC], mybir.dt.float32, name=f"out_sb_{it}", tag="out_sb"
    )

    if matmul == "serial":
        psums = [
            pspool.tile(
                [TOKENS, OBS], mybir.dt.float32, name=f"ps{it}_{ob}", tag=f"ps{ob}"
            )
            for ob in range(OB)
        ]
        for ki in range(KT):
            for ob in range(OB):
                nc.tensor.matmul(
                    psums[ob][:, :],
                    xs_sb[:, ki, :],
                    w_ap(ki, ob),
                    start=(ki == 0),
                    stop=(ki == KT - 1),
                )
        for ob in range(OB):
            nc.vector.tensor_copy(
                out_sb[:, ob * OBS : (ob + 1) * OBS], psums[ob][:, :]
            )
    elif matmul == "packed4":
        # 4 concurrent col-groups; group j accumulates k-tiles j mod 4 into
        # psum partitions [32j:32j+32]; folded 128->32 with 2 DVE adds.
        nrounds = KT // 4
        psums = [
            pspool.tile(
                [128, OBS], mybir.dt.float32, name=f"ps{it}_{ob}", tag=f"ps{ob}"
            )
            for ob in range(OB)
        ]
        fold = opool.tile(
            [64, OB, OBS], mybir.dt.float32, name=f"fold_{it}", tag="fold"
        )
        for r in range(nrounds):
            for j in range(4):
                ki = 4 * r + j
                for ob in range(OB):
                    nc.tensor.matmul(
                        psums[ob][32 * j : 32 * (j + 1), :],
                        xs_sb[:, ki, :],
                        w_ap(ki, ob),
                        start=(r == 0),
                        stop=(r == nrounds - 1),
                        tile_position=(0, 32 * j),
                        # sim's zero-region group check drops the partition
                        # base of col-group strips; disjoint strips are safe
                        skip_group_check=True,
                    )
        for ob in range(OB):
            nc.vector.tensor_add(
                fold[:, ob, :], psums[ob][0:64, :], psums[ob][64:128, :]
            )
            nc.vector.tensor_add(
                out_sb[:, ob * OBS : (ob + 1) * OBS],
                fold[0:32, ob, :],
                fold[32:64, ob, :],
            )
    else:
        raise ValueError(matmul)

    nc.sync.dma_start(out=out.ap(), in_=out_sb[:])


def _build(loops=1, hw_loop=False, deliver=DELIVER, matmul=MATMUL):
    key = (loops, hw_loop, deliver, matmul)
    if key in _cached_nc:
        return _cached_nc[key]

    nc = bacc.Bacc(
        "TRN2", target_bir_lowering=False, debug=False, num_devices=NCORES
    )
    xsT = nc.dram_tensor(
        "xsT", [IN_F, TOKENS], mybir.dt.bfloat16, kind="ExternalInput"
    )
    w_dt = mybir.dt.int8 if deliver == "cast" else mybir.dt.bfloat16
    wT = nc.dram_tensor("wT", [IN_F, OPC], w_dt, kind="ExternalInput")
    out = nc.dram_tensor(
        "out", [TOKENS, OPC], mybir.dt.float32, kind="ExternalOutput"
    )

    xsT_r = xsT.ap().rearrange("(nk p) t -> p nk t", p=128)  # [128, 32, 32]
    # row nk*512 + f*128 + p  ->  group nk, partition p, free (f, n)
    wT_r = wT.ap().rearrange("(nk f p) n -> nk p f n", f=KG, p=128)
    aps = (xsT_r, wT_r, out)

    with tile.TileContext(nc) as tc:
        with (
            tc.tile_pool(name="xpool", bufs=2) as xpool,
            tc.tile_pool(name="wpool", bufs=NG) as wpool,
            tc.tile_pool(name="opool", bufs=2) as opool,
            tc.tile_pool(name="pspool", bufs=2, space=bass.MemorySpace.PSUM) as pspool,
        ):
            pools = (xpool, wpool, opool, pspool)
            if hw_loop and loops > 1:
                with tc.For_i(0, loops, 1):
                    _emit_body(nc, pools, aps, deliver, matmul)
            else:
                for it in range(loops):
                    _emit_body(nc, pools, aps, deliver, matmul, it)

    nc.compile()
    _cached_nc[key] = nc
    return nc


def make_in_maps(x, weight, scales, deliver=DELIVER):
    x = np.asarray(x, dtype=np.float32)
    weight = np.asarray(weight)
    scales = np.asarray(scales, dtype=np.float32)
    assert x.shape == (TOKENS, IN_F) and weight.shape == (OUT_F, IN_F)

    xs = x * scales[None, :]
    xsT = np.ascontiguousarray(xs.T).astype(ml_dtypes.bfloat16)
    if deliver == "cast":
        wT = weight.astype(np.int8).T  # [IN_F, OUT_F] view
    else:
        wT = weight.astype(np.float32).T
    in_maps = []
    for c in range(NCORES):
        wTc = np.ascontiguousarray(wT[:, c * OPC : (c + 1) * OPC])
        if deliver != "cast":
            wTc = wTc.astype(ml_dtypes.bfloat16)
        in_maps.append({"xsT": xsT, "wT": wTc})
    return in_maps


def run(x, weight, scales, trace=False, trace_cores=None):
    nc = _build()
    in_maps = make_in_maps(x, weight, scales)
    res = run_bass_kernel_spmd(
        nc,
        in_maps,
        core_ids=list(range(NCORES)),
        trace=trace,
        trace_cores=trace_cores,
    )
    out = np.concatenate(
        [res.results[c]["out"] for c in range(NCORES)], axis=1
    ).astype(np.float32, copy=False)
    return out, res


def kernel(x, weight, scales):
    out, _ = run(x, weight, scales)
    return out
